# revision 108
# baseline (speedup 1.0000x reference)
"""MultiHeadAttention TRN2 Bass kernel (8 NeuronCores), fp8 DoubleRow version.

Sharding: core c = (batch b = c//2, query-half = c%2). Each core computes
K/V for its full batch (2048 keys) and attention + output projection + LN
for its 1024 query rows. No collectives; host gathers per-core outputs.

Device math (fp8e4m3 matmuls in DoubleRow perf mode = 2 k-tiles of 128
contracted per instruction at 0.5 cycles per output column):
  Q/K proj: out [hd(2 heads x 64dk), m]  = w8[d,2,hd].T @ qt8[d,2,m]
  V proj:   out [m, 2 heads x 64]        = qt8[d,2,m].T @ wv8[d,2,hv]
  kht2/qht2: DMA re-layout to [32, 2(dk-half), m] per head so S can run
  DoubleRow with dk = 2x32.
  S[m, q]   = kht2[32,2,mtile].T @ qht2[32,2,q]      (per head, per m-tile)
  E = exp(S/32): one [128,1024] two-bank PSUM tile per (head, m-tile),
      whole tiles routed to ACT (AF.Exp) or DVE (custom op
      (0.5*(1+x/8)^2+0.5)^8 ~ e^x, <1% typical error) by a greedy
      least-loaded balancer; Q/K/V PSUM->fp8 evacuations share the same
      two-engine budget (Pool/GPSIMD cannot touch PSUM on TRN2).
  O[q, 64]  = sum_p E[m,2,q].T @ V[m,2,64]           (E is the stationary)
  den[q]    = sum_p E[m,2,q].T @ ones[m,2,1]
  CT[q, (h,64)] = O * (1/den)    (per-partition scalar broadcast, DVE)
  ctT = DMA-transpose(CT bf16) -> fp8 (Pool, per quad, pipelined early);
  Y = ctT.T @ pw8 + residual (fused DVE add+row-sum); LayerNorm tail
  spread over ACT/DVE/Pool with the sqrt act-table preloaded before the
  qt pipeline.

Schedule: Q/K/V projection matmul chunks drip one-per-mt-slot through the
attention head loop (job queue) so the PE never bursts long enough to
drain the exp engines' 3-deep S-tile cushion; the previous head's O/den
matmuls are split into four 2-qt bursts for the same reason. Initial
DMAs fan out across the SP/ACT/Pool queues. When ln_a==1 and ln_b==0
(this module's configuration) the gamma/beta passes are skipped.
"""
import numpy as np
import ml_dtypes

import concourse.bass as bass
import concourse.mybir as mybir
import concourse.tile as tile
from concourse import bacc
from concourse.bass_utils import run_bass_kernel_spmd

# ---------------- custom DVE exp op ----------------
import concourse.dve_ops as dve_ops
from concourse.dve_spec import Spec, Src0, C0, C1, C2, sq


def _exp_ref(in0, in1, c0, c1, c2):
    x = in0.astype(np.float32)
    t = ((x * c0 + c1) ** 2 * c2 + c2).astype(np.float32)
    t = (t * t).astype(np.float32)
    t = (t * t).astype(np.float32)
    return (t * t).astype(np.float32)


def _get_exp_op():
    name = "EXP_APPROX_ANT"
    for op in dve_ops.OPS:
        if op.name == name:
            return op
    body = sq(sq(sq(sq(Src0 * C0 + C1) * C2 + C2)))
    op = dve_ops.DveOp(
        name,
        Spec(body=body, reference=_exp_ref),
        subdim=False,
        uops_sha={"v3": "7cc04e385f99d2ac", "v4": "4c6dc6b0499997cd"},
    )
    row = max(dve_ops._SUB_OPCODE_FOR_NAME.values()) + 1
    assert row < 0x20
    dve_ops.OPS.append(op)
    dve_ops.CUSTOM_DVE_SPECS[name] = op.spec
    dve_ops._SUB_OPCODE_FOR_NAME[name] = row
    return op


EXP_OP = _get_exp_op()


def _addred_ref(in0, in1, c0, c1, c2):
    b = (in0.astype(np.float32) + in1.astype(np.float32)).astype(np.float32)
    return b, b.reshape(b.shape[0], -1).sum(axis=-1, keepdims=True)


def _get_addred_op():
    from operator import add as _add
    from concourse.dve_spec import Src1, Zero
    name = "ADD_REDUCE_ANT"
    for op in dve_ops.OPS:
        if op.name == name:
            return op
    op = dve_ops.DveOp(
        name,
        Spec(body=Src0 + Src1, accum=_add, accum_init=Zero,
             reference=_addred_ref),
        subdim=False,
        uops_sha={"v3": "8be32207425579a6", "v4": "102f3739dc9078fe"},
    )
    row = max(dve_ops._SUB_OPCODE_FOR_NAME.values()) + 1
    assert row < 0x20
    dve_ops.OPS.append(op)
    dve_ops.CUSTOM_DVE_SPECS[name] = op.spec
    dve_ops._SUB_OPCODE_FOR_NAME[name] = row
    return op


ADDRED_OP = _get_addred_op()

F32 = mybir.dt.float32
BF16 = mybir.dt.bfloat16
F8 = mybir.dt.float8e4
AF = mybir.ActivationFunctionType
ALU = mybir.AluOpType
AX = mybir.AxisListType
PM = mybir.MatmulPerfMode

B, L, D = 4, 2048, 1024
H, DK = 16, 64
HALF = 1024
TEMPER = 32.0
LN_EPS = 1e-3
NP8 = ml_dtypes.float8_e4m3

_CACHE = {}


def build(iters=1, skip_gamma=False):
    nc = bacc.Bacc(None, target_bir_lowering=False)
    qt8_d = nc.dram_tensor("qt8", [128, 8, L], F8, kind="ExternalInput")
    wq8_d = nc.dram_tensor("wq8", [128, 8, H * DK], F8, kind="ExternalInput")
    wk8_d = nc.dram_tensor("wk8", [128, 8, H * DK], F8, kind="ExternalInput")
    wv8_d = nc.dram_tensor("wv8", [128, 8, H * DK], F8, kind="ExternalInput")
    pw8_d = nc.dram_tensor("pw8", [128, 8, D], F8, kind="ExternalInput")
    ones_d = nc.dram_tensor("ones8", [1, 2], F8, kind="ExternalInput")
    qres_d = nc.dram_tensor("qres", [HALF, D], BF16, kind="ExternalInput")
    lna_d = nc.dram_tensor("lna", [1, D], F32, kind="ExternalInput")
    lnb_d = nc.dram_tensor("lnb", [1, D], F32, kind="ExternalInput")
    out_d = nc.dram_tensor("out", [HALF, D], F32, kind="ExternalOutput")

    with tile.TileContext(nc) as tc:
        with (
            tc.tile_pool(name="c1", bufs=1) as c1,
            tc.tile_pool(name="st", bufs=2) as st,
            tc.tile_pool(name="eh", bufs=2) as ehp,
            tc.tile_pool(name="sm", bufs=2) as smp,
            tc.tile_pool(name="psD", bufs=1, space="PSUM") as psD,
            tc.tile_pool(name="psS", bufs=3, space="PSUM") as psS,
            tc.tile_pool(name="psO", bufs=1, space="PSUM") as psO,
        ):
            # persistent SBUF
            qt8 = c1.tile([128, 8, L], F8, name="qt8_t")
            wq8 = c1.tile([128, 8, H * DK], F8, name="wq8_t")
            wk8 = c1.tile([128, 8, H * DK], F8, name="wk8_t")
            wv8 = c1.tile([128, 8, H * DK], F8, name="wv8_t")
            pw8 = c1.tile([128, 8, D], F8, name="pw8_t")
            ones8 = c1.tile([128, 2, 1], F8, name="ones8_t")
            kht2 = c1.tile([128, 6, 2, L], F8, name="kht2_t")
            qht2 = c1.tile([128, 6, 2, HALF], F8, name="qht2_t")
            vaug = c1.tile([128, 16, H * DK], F8, name="vaug_t")
            ctT8 = c1.tile([128, 8, D], F8, name="ctT8_t")
            if not skip_gamma:
                lna_t = c1.tile([128, D], F32, name="lna_t")
                lnb_t = c1.tile([128, D], F32, name="lnb_t")

            nc.sync.dma_start(qt8[:, :, 0:512], qt8_d[:, :, 0:512])
            nc.gpsimd.dma_start(qt8[:, :, 512:1024], qt8_d[:, :, 512:1024])

            for it in range(iters):
                P = f"it{it}_"

                jobs = []

                def run_job():
                    if jobs:
                        jobs.pop(0)()

                # greedy ACT/DVE balancer: route each balanceable op to the
                # engine with less emitted work (costs in ns, cost-model
                # calibrated); fixed ops charge their engine explicitly
                eng_ns = {"act": 0.0, "dve": 0.0}

                def balanced(cost_act, cost_dve, f_act, f_dve):
                    if eng_ns["act"] + cost_act <= eng_ns["dve"] + cost_dve:
                        eng_ns["act"] += cost_act
                        f_act()
                    else:
                        eng_ns["dve"] += cost_dve
                        f_dve()

                def evac(dst, src, n):
                    """PSUM->SBUF fp8 conversion, ACT or DVE by load."""
                    balanced(
                        n * 0.833 + 185, n * 1.042 + 125,
                        lambda: nc.scalar.activation(dst, src, AF.Copy),
                        lambda: nc.vector.tensor_copy(dst, src))

                def qk_job(hp, w8, wd, ncols, dst2, name, split_dst=False):
                    """Closures projecting K (ncols=L) or Q (ncols=HALF) for
                    head-pair hp: per-1024 matmul+evac chunks, then DMA
                    re-layout into dst2's [32, 2(dk-half), m] layout.
                    split_dst re-layouts after every chunk (lower latency)."""
                    chunk = st.tile([128, ncols], F8, name=f"{P}{name}c_{hp}",
                                    tag=f"{name}stage")

                    def wdma():
                        nc.sync.dma_start(w8[:, :, hp * 128:(hp + 1) * 128],
                                          wd[:, :, hp * 128:(hp + 1) * 128])

                    def relayout(c0, c1):
                        for h2 in range(2):
                            h = 2 * hp + h2
                            for t in range(2):
                                nc.sync.dma_start(
                                    dst2[32 * (h % 3):32 * (h % 3) + 32,
                                         h // 3, t, c0:c1],
                                    chunk[64 * h2 + 32 * t:
                                          64 * h2 + 32 * t + 32, c0:c1],
                                )

                    def mk_chunk(mc):
                        def f():
                            ps = psS.tile([128, 1024], F32,
                                          name=f"{P}{name}ps_{hp}_{mc}",
                                          tag="sa")
                            for cc in range(4):
                                for j in range(4):
                                    nc.tensor.matmul(
                                        ps[:, cc * 256:(cc + 1) * 256],
                                        w8[:, 2 * j:2 * j + 2,
                                           hp * 128:(hp + 1) * 128],
                                        qt8[:, 2 * j:2 * j + 2,
                                            mc * 1024 + cc * 256:
                                            mc * 1024 + (cc + 1) * 256],
                                        start=(j == 0), stop=(j == 3),
                                        perf_mode=PM.DoubleRow,
                                    )
                            evac(chunk[:, mc * 1024:(mc + 1) * 1024], ps[:],
                                 1024)
                            if split_dst:
                                relayout(mc * 1024, (mc + 1) * 1024)
                            elif mc == ncols // 1024 - 1:
                                relayout(0, ncols)
                        return f
                    return [wdma] + [mk_chunk(mc) for mc in range(ncols // 1024)]

                def v_job(mt):
                    """V for m-tile mt, all 16 heads in one chunk."""
                    def f():
                        ps = psS.tile([128, 1024], F32,
                                      name=f"{P}vps_{mt}", tag="sa")
                        for hp in range(8):
                            for j in range(4):
                                nc.tensor.matmul(
                                    ps[:, hp * 128:(hp + 1) * 128],
                                    qt8[:, 2 * j:2 * j + 2,
                                        mt * 128:(mt + 1) * 128],
                                    wv8[:, 2 * j:2 * j + 2,
                                        hp * 128:(hp + 1) * 128],
                                    start=(j == 0), stop=(j == 3),
                                    perf_mode=PM.DoubleRow,
                                )
                        evac(vaug[:, mt, :], ps[:], 1024)
                    return f

                ehs = {}
                ots = {}
                dens = {}
                recbs = {}

                def attn_ov(h, qts=range(8)):
                    """O + den matmuls for head h (PE), emitted one head late
                    so the division pipeline never blocks the engines."""
                    eh, ot, den = ehs[h], ots[h], dens[h]
                    for qt in qts:
                        for p in range(8):
                            nc.tensor.matmul(
                                ot[:, qt, :],
                                eh[:, p, :, qt * 128:(qt + 1) * 128],
                                vaug[:, 2 * p:2 * p + 2, h * 64:(h + 1) * 64],
                                start=(p == 0), stop=(p == 7),
                                perf_mode=PM.DoubleRow,
                            )
                            # same stationary as the ot matmul above -> the
                            # PE can skip the weight reload
                            nc.tensor.matmul(
                                den[:, qt:qt + 1],
                                eh[:, p, :, qt * 128:(qt + 1) * 128],
                                ones8[:, :, 0:1],
                                start=(p == 0), stop=(p == 7),
                                perf_mode=PM.DoubleRow,
                            )

                def attn_recb(h):
                    recb = smp.tile([128, 8], F32, name=f"{P}rec_{h}",
                                    tag="recb")
                    recbs[h] = recb
                    nc.vector.reciprocal_approx_fast(recb[:], dens[h][:])

                cbq = [None]
                ctqs = {}

                def attn_div(h):
                    """CT = O * (1/den) in ONE stride-0-broadcast Pool op per
                    head, into a 4-head staging block; transpose per quad.
                    Completed quads are converted bf16->fp8 early (Pool)."""
                    ot, recb = ots[h], recbs[h]
                    if h % 4 == 0:
                        cbq[0] = st.tile([128, 8, 4, 64], BF16,
                                         name=f"{P}cbq_{h // 4}",
                                         tag="ctblk", bufs=2)
                    cb = cbq[0]
                    eng_ns["dve"] += 727
                    if h == 15:
                        # final head: divide in qt-halves so the first
                        # transposes (tail critical path) start earlier
                        nc.vector.tensor_tensor(
                            cb[:, 0:4, 3, :], ot[:, 0:4, :],
                            recb[:, 0:4].to_broadcast([128, 4, 64]), ALU.mult)
                    else:
                        nc.vector.tensor_tensor(
                            cb[:, :, h % 4, :], ot[:, :, :],
                            recb[:].to_broadcast([128, 8, 64]), ALU.mult)
                    if h % 4 == 3:
                        q4 = h // 4
                        ctq = st.tile([128, 2, D], BF16,
                                      name=f"{P}ctq_{q4}", tag="ctq", bufs=2)
                        ctqs[q4] = ctq
                        for qt in range(8):
                            if h == 15 and qt == 4:
                                nc.vector.tensor_tensor(
                                    cb[:, 4:8, 3, :], ot[:, 4:8, :],
                                    recb[:, 4:8].to_broadcast([128, 4, 64]),
                                    ALU.mult)
                            # for the final quad, split across two queues to
                            # halve the serial transpose latency in the tail
                            eng_t = nc.scalar if (h == 15 and qt % 2) \
                                else nc.sync
                            eng_t.dma_start_transpose(
                                ctq[:, :, qt * 128:(qt + 1) * 128],
                                cb[:, qt, :, :])
                    if h % 4 == 3 and h >= 7:
                        q4 = h // 4 - 1
                        nc.gpsimd.tensor_copy(
                            ctT8[:, 2 * q4:2 * q4 + 2, :], ctqs[q4][:])

                def attn_sx(h):
                    """S matmuls + one whole-tile exp per (h, mt), alternating
                    ACT/DVE; interleaves the previous head's O/den/recb/div
                    and one pending projection chunk per mt."""
                    a, b2 = h % 3, h // 3
                    eh = ehp.tile([128, 8, 2, HALF], F8, name=f"{P}eh_{h}",
                                  tag="eh")
                    ehs[h] = eh
                    ots[h] = psO.tile([128, 8, 64], F32, name=f"{P}ot_{h}",
                                      tag="ot")
                    dens[h] = psD.tile([128, 8], F32, name=f"{P}den_{h}",
                                       tag="den")
                    for mt in range(16):
                        run_job()
                        sp = psS.tile([128, 1024], F32,
                                      name=f"{P}sp_{h}_{mt}", tag="sa")
                        for qc in range(4):
                            nc.tensor.matmul(
                                sp[:, qc * 256:(qc + 1) * 256],
                                kht2[32 * a:32 * a + 32, b2, :,
                                     mt * 128:(mt + 1) * 128],
                                qht2[32 * a:32 * a + 32, b2, :,
                                     qc * 256:(qc + 1) * 256],
                                start=True, stop=True,
                                perf_mode=PM.DoubleRow,
                            )
                        dst = eh[:, mt // 2, mt % 2, :]
                        balanced(
                            1049, 1192,
                            lambda: nc.scalar.activation(
                                dst, sp[:], AF.Exp, scale=1.0 / TEMPER),
                            lambda: nc.vector._custom_dve(
                                EXP_OP, out=dst, in0=sp[:],
                                s0=1.0 / (TEMPER * 8.0), s1=1.0, imm2=0.5))
                        # split the previous head's O/den matmuls into four
                        # short bursts so the PE never drains the exp
                        # engines' S-tile cushion. Head 0's O needs the full
                        # vaug (still streaming during head 1), so its bursts
                        # sit at the very end of head 1.
                        if h == 15:
                            # final head: finish head 14's O/recip/div inside
                            # the loop so the psO bank frees before ov(15)
                            if mt in (7, 9, 11, 13):
                                attn_ov(14, range(mt - 7, mt - 5))
                            elif mt == 14:
                                attn_recb(14)
                            elif mt == 15:
                                attn_div(14)
                        elif h >= 2 and mt in (9, 11, 13, 15):
                            attn_ov(h - 1, range(mt - 9, mt - 7))
                        elif h == 1 and mt in (12, 13, 14, 15):
                            attn_ov(0, range((mt - 12) * 2, (mt - 12) * 2 + 2))
                    if 1 <= h <= 14:
                        attn_recb(h - 1)
                        attn_div(h - 1)

                def attn_last(h):
                    # O/den + reciprocal in qt-halves: the first division
                    # half fires while the PE still runs qt 4-7's O matmuls
                    recb = smp.tile([128, 8], F32, name=f"{P}rec_{h}",
                                    tag="recb")
                    recbs[h] = recb
                    attn_ov(h, range(0, 4))
                    nc.vector.reciprocal_approx_fast(recb[:, 0:4],
                                                     dens[h][:, 0:4])
                    attn_ov(h, range(4, 8))
                    nc.vector.reciprocal_approx_fast(recb[:, 4:8],
                                                     dens[h][:, 4:8])
                    attn_div(h)

                qrs = {}

                def load_qr(qt):
                    qr = st.tile([128, D], BF16, name=f"{P}qr_{qt}",
                                 tag="qr", bufs=8)
                    qrs[qt] = qr
                    nc.sync.dma_start(qr[:], qres_d[qt * 128:(qt + 1) * 128, :])

                # ---- emission schedule: head 0's Q/K projected up front;
                # all other projections drip one chunk per mt slot ----------
                # initial loads fan out over idle engine DMA queues so the
                # first projection isn't serialized behind one queue
                nc.scalar.dma_start(wq8[:, :, 0:128], wq8_d[:, :, 0:128])
                nc.sync.dma_start(wk8[:, :, 0:128], wk8_d[:, :, 0:128])
                nc.sync.dma_start(qt8[:, :, 1024:1536],
                                  qt8_d[:, :, 1024:1536])
                nc.gpsimd.dma_start(qt8[:, :, 1536:2048],
                                    qt8_d[:, :, 1536:2048])
                qjob = qk_job(0, wq8, wq8_d, HALF, qht2, "q")
                kjob = qk_job(0, wk8, wk8_d, L, kht2, "k", split_dst=True)
                for f in qjob[1:]:
                    f()
                kjob[1]()
                # head-0 re-layout DMAs now precede these non-critical loads
                # on the SP queue
                nc.sync.dma_start(wv8[:, :, 0:512], wv8_d[:, :, 0:512])
                nc.gpsimd.dma_start(wv8[:, :, 512:1024], wv8_d[:, :, 512:1024])
                nc.sync.dma_start(ones8[:, :, 0],
                                  ones_d[:].to_broadcast([128, 2]))
                kjob[2]()

                for h in range(16):
                    if h == 0:
                        for mt in range(8):
                            jobs.append(v_job(mt))
                            jobs.append(lambda: None)
                    elif h == 1:
                        q1 = qk_job(1, wq8, wq8_d, HALF, qht2, "q")
                        k1 = qk_job(1, wk8, wk8_d, L, kht2, "k")
                        v1 = [v_job(mt) for mt in range(8, 16)]
                        jobs.extend([k1[0], v1[0], k1[1], v1[1], k1[2],
                                     v1[2], q1[0], v1[3], q1[1], v1[4],
                                     v1[5], v1[6], v1[7]])
                    elif 2 <= h <= 7:
                        jobs.extend(qk_job(h, wq8, wq8_d, HALF, qht2, "q"))
                        jobs.extend(qk_job(h, wk8, wk8_d, L, kht2, "k"))
                    if h == 6:
                        def late_dmas():
                            nc.sync.dma_start(pw8[:], pw8_d[:])
                            if not skip_gamma:
                                nc.sync.dma_start(
                                    lna_t[:], lna_d[:].to_broadcast([128, D]))
                                nc.sync.dma_start(
                                    lnb_t[:], lnb_d[:].to_broadcast([128, D]))
                        jobs.append(late_dmas)
                    if h == 13:
                        for qt in range(8):
                            load_qr(qt)
                    attn_sx(h)
                while jobs:
                    run_job()
                # preload the sqrt act table while head 15's O matmuls run;
                # input reads the last exp tile so the scheduler cannot hoist
                # this above the exp stream (which would thrash the table)
                warm = smp.tile([128, 1], F32, name=f"{P}warm", tag="ln_sg")
                nc.scalar.activation(warm[:], ehs[15][:, 7, 1, 0:1], AF.Sqrt)
                attn_last(15)

                # quad-3 ctTb -> fp8 (quads 0-2 were converted in attn_div);
                # per-qt-pair so the first projections start sooner. The
                # first two pairs ride the otherwise-idle DVE.
                for i in range(4):
                    eng_c = nc.vector if i < 2 else nc.gpsimd
                    eng_c.tensor_copy(
                        ctT8[:, 6:8, i * 256:(i + 1) * 256],
                        ctqs[3][:, :, i * 256:(i + 1) * 256])
                for qt in range(8):
                    qr = qrs[qt]
                    yt = st.tile([128, D], F32, name=f"{P}yt_{qt}", tag="yt")
                    sh = smp.tile([128, 2], F32, name=f"{P}sh_{qt}", tag="ln_sh")
                    yp = psS.tile([128, 1024], F32, name=f"{P}yp_{qt}",
                                  tag="sa")
                    for oc in range(2):
                        for cc in range(2):
                            for j in range(4):
                                nc.tensor.matmul(
                                    yp[:, oc * 512 + cc * 256:
                                       oc * 512 + (cc + 1) * 256],
                                    ctT8[:, 2 * j:2 * j + 2,
                                         qt * 128:(qt + 1) * 128],
                                    pw8[:, 2 * j:2 * j + 2,
                                        oc * 512 + cc * 256:
                                        oc * 512 + (cc + 1) * 256],
                                    start=(j == 0), stop=(j == 3),
                                    perf_mode=PM.DoubleRow,
                                )
                    for oc in range(2):
                        # fused residual add + row-sum accumulate
                        eng_ns["dve"] += 658
                        nc.vector._custom_dve(
                            ADDRED_OP, out=yt[:, oc * 512:(oc + 1) * 512],
                            in0=yp[:, oc * 512:(oc + 1) * 512],
                            in1=qr[:, oc * 512:(oc + 1) * 512],
                            accum_out=sh[:, oc:oc + 1])
                    # layernorm: mu, sigma (ddof=1), (y-mu)/(sigma+eps)*a+b
                    o_t = st.tile([128, D], F32, name=f"{P}o_{qt}", tag="o")
                    s = smp.tile([128, 1], F32, name=f"{P}s_{qt}", tag="ln_s")
                    nc.vector.tensor_add(s[:], sh[:, 0:1], sh[:, 1:2])
                    negmean = smp.tile([128, 1], F32, name=f"{P}nm_{qt}",
                                       tag="ln_nm")
                    nc.vector.tensor_scalar_mul(negmean[:], s[:], -1.0 / D)
                    ss = smp.tile([128, 1], F32, name=f"{P}ss_{qt}", tag="ln_ss")
                    # scratch output: only the accumulated row-sum is used;
                    # a dedicated tile keeps the o_t/out-DMA chain off the
                    # squares' critical path
                    sqs = st.tile([128, D], F32, name=f"{P}sqs_{qt}",
                                  tag="sqscratch", bufs=1)
                    eng_ns["act"] += 1330
                    nc.scalar.activation(sqs[:], yt[:], AF.Square,
                                         bias=negmean[:], accum_out=ss[:])
                    sigma = smp.tile([128, 1], F32, name=f"{P}sg_{qt}",
                                     tag="ln_sg")
                    nc.scalar.activation(sigma[:], ss[:], AF.Sqrt,
                                         scale=1.0 / (D - 1))
                    dd = smp.tile([128, 1], F32, name=f"{P}dd_{qt}", tag="ln_dd")
                    nc.vector.tensor_scalar_add(dd[:], sigma[:], LN_EPS)
                    rec2 = smp.tile([128, 1], F32, name=f"{P}rc_{qt}",
                                    tag="ln_rc")
                    nc.vector.reciprocal_approx_fast(rec2[:], dd[:])
                    if qt in (0, 1, 7):
                        eng_ns["dve"] += 594
                        nc.vector.tensor_scalar(o_t[:], yt[:], negmean[:],
                                                rec2[:], ALU.add, ALU.mult)
                    else:
                        nc.gpsimd.tensor_scalar(o_t[:], yt[:], negmean[:],
                                                rec2[:], ALU.add, ALU.mult)
                    if not skip_gamma:
                        nc.gpsimd.tensor_mul(o_t[:], o_t[:], lna_t[:])
                        nc.gpsimd.tensor_add(o_t[:], o_t[:], lnb_t[:])
                    # the last stores issue from otherwise-idle queues so
                    # their descriptors don't serialize behind each other
                    if qt >= 6:
                        # last two stores split in halves across idle DMA
                        # queues: descriptors and transfers run in parallel
                        e0, e1 = ((nc.sync, nc.gpsimd) if qt == 6
                                  else (nc.scalar, nc.sync))
                        e0.dma_start(out_d[qt * 128:(qt + 1) * 128, 0:512],
                                     o_t[:, 0:512])
                        e1.dma_start(out_d[qt * 128:(qt + 1) * 128, 512:D],
                                     o_t[:, 512:D])
                    else:
                        nc.sync.dma_start(out_d[qt * 128:(qt + 1) * 128, :],
                                          o_t[:])

    nc.compile()
    return nc


def _get_nc(skip_gamma=False):
    key = f"nc_{skip_gamma}"
    if key not in _CACHE:
        _CACHE[key] = build(skip_gamma=skip_gamma)
    return _CACHE[key]


def _interleave8(a):
    """[D, N] f32 -> [128, 8, N] fp8 with [p, 2j+t, n] = a[j*256+t*128+p, n]."""
    n = a.shape[1]
    return np.ascontiguousarray(
        a.reshape(4, 2, 128, n).transpose(2, 0, 1, 3).reshape(128, 8, n)
    ).astype(NP8)


def _in_maps(q, w_qs, w_ks, w_vs, proj_w, proj_b, ln_a, ln_b):
    wq8 = _interleave8(np.ascontiguousarray(
        w_qs.transpose(1, 0, 2).reshape(D, H * DK)))
    wk8 = _interleave8(np.ascontiguousarray(
        w_ks.transpose(1, 0, 2).reshape(D, H * DK)))
    wv8 = _interleave8(np.ascontiguousarray(
        w_vs.transpose(1, 0, 2).reshape(D, H * DK)))
    pw8 = _interleave8(np.ascontiguousarray(proj_w.T))
    ones8 = np.ones((1, 2), NP8)
    lna = np.ascontiguousarray(ln_a[None, :]).astype(np.float32)
    lnb = np.ascontiguousarray(ln_b[None, :]).astype(np.float32)
    maps = []
    for c in range(8):
        b, half = c // 2, c % 2
        qb = q[b]
        perm = np.r_[half * HALF:(half + 1) * HALF,
                     (1 - half) * HALF:(2 - half) * HALF]
        qt8 = _interleave8(np.ascontiguousarray(qb.T[:, perm]))
        qres = np.ascontiguousarray(
            qb[half * HALF:(half + 1) * HALF, :] + proj_b[None, :]
        ).astype(ml_dtypes.bfloat16)
        maps.append({
            "qt8": qt8, "qres": qres,
            "wq8": wq8, "wk8": wk8, "wv8": wv8, "pw8": pw8,
            "ones8": ones8, "lna": lna, "lnb": lnb,
        })
    return maps


def kernel(q, w_qs, w_ks, w_vs, proj_w, proj_b, ln_a, ln_b, **kw):
    q = np.asarray(q, dtype=np.float32)
    w_qs = np.asarray(w_qs, dtype=np.float32)
    w_ks = np.asarray(w_ks, dtype=np.float32)
    w_vs = np.asarray(w_vs, dtype=np.float32)
    proj_w = np.asarray(proj_w, dtype=np.float32)
    proj_b = np.asarray(proj_b, dtype=np.float32)
    ln_a = np.asarray(ln_a, dtype=np.float32)
    ln_b = np.asarray(ln_b, dtype=np.float32)

    in_maps = _in_maps(q, w_qs, w_ks, w_vs, proj_w, proj_b, ln_a, ln_b)
    # identity affine (the common case) skips the gamma/beta passes on-device
    skip_gamma = bool(np.all(ln_a == 1.0) and np.all(ln_b == 0.0))
    nc = _get_nc(skip_gamma=skip_gamma)
    _CACHE["last"] = nc
    res = run_bass_kernel_spmd(nc, in_maps, core_ids=list(range(8))).results

    out = np.empty((B, L, D), dtype=np.float32)
    for c in range(8):
        b, half = c // 2, c % 2
        out[b, half * HALF:(half + 1) * HALF, :] = res[c]["out"]
    return out

